# revision 1
# baseline (speedup 1.0000x reference)
"""Trainium2 Bass kernel for nn_NodeEncoding_72816875537095.

Reference computation:
    scores = x @ W[0] + b[0]                          # [total]
    sp     = scatter(scores, pad_idx) -> [B, 96]      # padded per-graph scores
    num    = einsum('bijk,bk->bij', paths, sp)
    den    = paths.sum(-1) + 1e-8
    out    = num / den                                # [64, 96, 96]

Strategy (data-parallel over B across 8 NeuronCores, 8 graphs/core):
  - Host relayout: per core+graph, paths -> k-major [128, 9216] bf16 tiles
    (k rows 96..127 zero-padded).  0/1 path values are exact in bf16, the
    pad fills all 128 SBUF partitions (measured 329 GB/s vs 188 GB/s for
    96-partition DMAs), and bf16 halves the bytes.
  - Device: per 128-column chunk of a graph, ONE matmul with the paths
    chunk as the bf16 stationary operand [128(k), 128(ij)] (fast weight
    load) and a 4-column moving operand [sp_hi, sp_lo, ones, 0] -> PSUM
    [128, 4] = (num_hi, num_lo, den, -) for 128 output elements.  sp is
    hi/lo bf16-split for near-fp32 accuracy.  The paths data streams
    through the PE exactly once.
  - 128 chunks pack into one PSUM bank side by side; epilogue per bank is
    a handful of wide strided ops: num = hi+lo (DVE), den+eps (ScalarE
    copy), reciprocal (DVE), multiply (DVE).
  - Output is stored partition-major [128, 576]; host un-permutes.
"""

import sys

if "/opt/trn_rl_repo" not in sys.path:
    sys.path.insert(0, "/opt/trn_rl_repo")

import ml_dtypes
import numpy as np

import concourse.bass as bass  # noqa: F401
import concourse.mybir as mybir
from concourse import bacc, bass_utils
from concourse.tile import TileContext

F32 = mybir.dt.float32
BF16 = mybir.dt.bfloat16
FP8 = mybir.dt.float8e4
AF = mybir.ActivationFunctionType

B = 64
MAX_A = 96
D = 256
N_CORES = 8
G = B // N_CORES            # 8 graphs per core
COLS = MAX_A * MAX_A        # 9216
KP = 128                    # padded contraction rows
CHUNK = 128                 # stationary columns per matmul
CPG = COLS // CHUNK         # 72 chunks per graph
TOT = G * CPG               # 576 chunks per core
CPT = 128                   # chunks per PSUM tile (128*4 = 512 cols = 1 bank)
EPS = 1e-8

_NC_CACHE = {}


def _build():
    if "nc" in _NC_CACHE:
        return _NC_CACHE["nc"]

    nc = bacc.Bacc("TRN2", target_bir_lowering=False, debug=False,
                   num_devices=N_CORES)

    pathsT_d = nc.dram_tensor("pathsT", [G, KP, COLS], FP8,
                              kind="ExternalInput")
    xg_d = nc.dram_tensor("xg", [MAX_A, G * D], F32, kind="ExternalInput")
    wrep_d = nc.dram_tensor("wrep", [MAX_A, G * D], F32, kind="ExternalInput")
    bmask_d = nc.dram_tensor("bmask", [MAX_A, G], F32, kind="ExternalInput")
    out_d = nc.dram_tensor("out", [CHUNK, TOT], F32, kind="ExternalOutput")

    with TileContext(nc) as tc:
        with (
            tc.tile_pool(name="misc", bufs=1) as misc,
            tc.tile_pool(name="paths", bufs=4) as ppool,
            tc.tile_pool(name="psum", bufs=2, space="PSUM") as pspool,
            tc.tile_pool(name="epi", bufs=3) as epool,
        ):
            # Pre-issue the first paths supertile loads so the big DMAs
            # start immediately (the scores inputs ride the SWDGE queue).
            head_tiles = {}
            for g in range(min(4, G)):
                st = ppool.tile([KP, COLS], FP8, tag="st", name=f"st{g}")
                nc.sync.dma_start(out=st[:], in_=pathsT_d[g])
                head_tiles[g] = st

            # ---- node scores -> w_all [128(k), 4*G] bf16 ----
            xt = misc.tile([MAX_A, G * D], F32)
            nc.scalar.dma_start(out=xt[:], in_=xg_d[:])
            wr = misc.tile([MAX_A, G * D], F32)
            nc.scalar.dma_start(out=wr[:], in_=wrep_d[:])
            bm = misc.tile([MAX_A, G], F32)
            nc.scalar.dma_start(out=bm[:], in_=bmask_d[:])

            prod = misc.tile([MAX_A, G * D], F32)
            nc.vector.tensor_tensor(out=prod[:], in0=xt[:], in1=wr[:],
                                    op=mybir.AluOpType.mult)
            raw = misc.tile([MAX_A, G], F32)
            nc.vector.tensor_reduce(
                out=raw[:], in_=prod[:].rearrange("p (g d) -> p g d", d=D),
                axis=mybir.AxisListType.X, op=mybir.AluOpType.add)
            w_sp = misc.tile([MAX_A, G], F32)
            nc.vector.tensor_tensor(out=w_sp[:], in0=raw[:], in1=bm[:],
                                    op=mybir.AluOpType.add)
            w_hi = misc.tile([MAX_A, G], FP8)
            nc.vector.tensor_copy(w_hi[:], w_sp[:])
            r1 = misc.tile([MAX_A, G], F32)
            nc.vector.tensor_tensor(out=r1[:], in0=w_sp[:], in1=w_hi[:],
                                    op=mybir.AluOpType.subtract)
            w_lo1 = misc.tile([MAX_A, G], FP8)
            nc.vector.tensor_scalar_mul(out=w_lo1[:], in0=r1[:],
                                        scalar1=16.0)
            r2 = misc.tile([MAX_A, G], F32)
            nc.vector.scalar_tensor_tensor(
                out=r2[:], in0=w_lo1[:], scalar=-0.0625, in1=r1[:],
                op0=mybir.AluOpType.mult, op1=mybir.AluOpType.add)
            w_lo2 = misc.tile([MAX_A, G], FP8)
            nc.vector.tensor_scalar_mul(out=w_lo2[:], in0=r2[:],
                                        scalar1=256.0)

            # moving operand: per graph g, columns [4g..4g+4) =
            # [sp_hi, sp_lo1*16, sp_lo2*256, ones]; rows 96..127 zero.
            w_all = misc.tile([KP, 4 * G], FP8)
            nc.vector.memset(w_all[:], 0.0)
            nc.vector.memset(w_all[:, 3:4 * G:4], 1.0)
            nc.vector.tensor_copy(w_all[0:MAX_A, 0:4 * G:4], w_hi[:])
            nc.vector.tensor_copy(w_all[0:MAX_A, 1:4 * G:4], w_lo1[:])
            nc.vector.tensor_copy(w_all[0:MAX_A, 2:4 * G:4], w_lo2[:])

            out_sb = misc.tile([CHUNK, TOT], F32)

            # ---- main loop: one matmul per 128-column chunk ----
            ps = None
            for g in range(G):
                if g in head_tiles:
                    st = head_tiles[g]
                else:
                    st = ppool.tile([KP, COLS], FP8, tag="st",
                                    name=f"st{g}")
                    nc.sync.dma_start(out=st[:], in_=pathsT_d[g])
                for cl in range(CPG):
                    c = CPG * g + cl
                    r = c % CPT
                    if r == 0:
                        n_in_tile = min(CPT, TOT - c)
                        ps = pspool.tile([CHUNK, 4 * n_in_tile], F32,
                                         tag="ps")
                    nc.tensor.matmul(
                        ps[:, 4 * r:4 * r + 4],
                        lhsT=st[:, CHUNK * cl:CHUNK * (cl + 1)],
                        rhs=w_all[:, 4 * g:4 * g + 4],
                        start=True, stop=True)
                    if r == n_in_tile - 1:
                        t0 = c // CPT
                        w = n_in_tile
                        hi_sb = epool.tile([CHUNK, CPT], F32, tag="hi")
                        nc.scalar.activation(
                            out=hi_sb[:, :w], in_=ps[:, 0:4 * w:4],
                            func=AF.Copy)
                        t1 = epool.tile([CHUNK, CPT], F32, tag="t1")
                        nc.vector.scalar_tensor_tensor(
                            out=t1[:, :w], in0=ps[:, 1:4 * w:4],
                            scalar=0.0625, in1=hi_sb[:, :w],
                            op0=mybir.AluOpType.mult,
                            op1=mybir.AluOpType.add)
                        numt = epool.tile([CHUNK, CPT], F32, tag="numt")
                        nc.vector.scalar_tensor_tensor(
                            out=numt[:, :w], in0=ps[:, 2:4 * w:4],
                            scalar=0.00390625, in1=t1[:, :w],
                            op0=mybir.AluOpType.mult,
                            op1=mybir.AluOpType.add)
                        den_sb = epool.tile([CHUNK, CPT], F32, tag="den")
                        nc.scalar.activation(
                            out=den_sb[:, :w], in_=ps[:, 3:4 * w:4],
                            func=AF.Copy, bias=EPS)
                        rec = epool.tile([CHUNK, CPT], F32, tag="rec")
                        nc.vector.reciprocal(out=rec[:, :w],
                                             in_=den_sb[:, :w])
                        nc.vector.tensor_tensor(
                            out=out_sb[:, CPT * t0:CPT * t0 + w],
                            in0=numt[:, :w], in1=rec[:, :w],
                            op=mybir.AluOpType.mult)

            nc.sync.dma_start(out=out_d[:], in_=out_sb[:])

    nc.compile()
    _NC_CACHE["nc"] = nc
    return nc


def _host_prep(x, W, b, paths, pad_idx):
    x = np.ascontiguousarray(np.asarray(x, dtype=np.float32))
    W = np.asarray(W, dtype=np.float32)
    b = np.asarray(b, dtype=np.float32)
    pad_idx = np.asarray(pad_idx)

    xsc = np.zeros((B * MAX_A, D), dtype=np.float32)
    xsc[pad_idx] = x
    valid = np.zeros((B * MAX_A,), dtype=np.float32)
    valid[pad_idx] = 1.0
    bmask_full = (b[0] * valid).reshape(B, MAX_A)

    wrep = np.ascontiguousarray(np.tile(W.reshape(1, D), (MAX_A, G)))

    paths_bf = np.asarray(paths).astype(ml_dtypes.float8_e4m3)

    in_maps = []
    for core in range(N_CORES):
        g0 = core * G
        pc = paths_bf[g0:g0 + G]  # [G, 96, 96, 96] bf16
        pathsT = np.zeros((G, KP, COLS), dtype=ml_dtypes.float8_e4m3)
        pathsT[:, :MAX_A, :] = pc.transpose(0, 3, 1, 2).reshape(
            G, MAX_A, COLS)
        xc = np.ascontiguousarray(
            xsc[g0 * MAX_A:(g0 + G) * MAX_A]
            .reshape(G, MAX_A, D).transpose(1, 0, 2).reshape(MAX_A, G * D))
        bmask = np.ascontiguousarray(bmask_full[g0:g0 + G].T)
        in_maps.append({
            "pathsT": pathsT,
            "xg": xc,
            "wrep": wrep,
            "bmask": bmask,
        })
    return in_maps


LAST_RESULTS = None


def kernel(x, W, b, paths, pad_idx, _trace=False):
    global LAST_RESULTS
    nc = _build()
    in_maps = _host_prep(x, W, b, paths, pad_idx)
    res = bass_utils.run_bass_kernel_spmd(
        nc, in_maps, core_ids=list(range(N_CORES)), trace=_trace)
    LAST_RESULTS = res

    out = np.empty((B, MAX_A, MAX_A), dtype=np.float32)
    for core in range(N_CORES):
        oc = res.results[core]["out"]  # [128, 576] partition-major
        out[core * G:(core + 1) * G] = oc.T.reshape(G, MAX_A, MAX_A)
    return out



# revision 5
# speedup vs baseline: 1.3323x; 1.3323x over previous
"""Trainium2 Bass kernel for nn_NodeEncoding_72816875537095.

Reference computation:
    scores = x @ W[0] + b[0]                          # [total]
    sp     = scatter(scores, pad_idx) -> [B, 96]      # padded per-graph scores
    num    = einsum('bijk,bk->bij', paths, sp)
    den    = paths.sum(-1) + 1e-8
    out    = num / den                                # [64, 96, 96]

Strategy (data-parallel over B across 8 NeuronCores, 8 graphs/core):
  - paths (0/1 valued) are exact in fp8e4m3.  Per graph, the k-major
    [96, 9216] matrix is cut into 72 column-chunks of 128.  "Fold-32"
    packing: chunks are grouped 4-per-3-tiles; stationary tile t of a
    group holds chunk (4q+t) k-rows on partitions 0..95 AND a 32-row
    k-slice [32t, 32t+32) of chunk (4q+3) on partitions 96..127.  Every
    PE cell carries real data: 432 ldweights/core (the minimum) and
    7.08 MB of HBM traffic/core (no zero padding).
  - Moving operand per tile: 6 fp8 columns
        [sp_hi, sp_lo, ones (rows 0..95) | fold sp_hi, sp_lo, ones
         (rows 96..127, k-slice t of the folded chunk)]
    -> PSUM [128, 6] = main (num_hi, num_lo, den) + folded partials.
    sp is hi/lo fp8-split (8+ mantissa bits); folded chunks sum their 3
    partials with one DVE tensor_reduce per psum bank.
  - node scores run on the PE too: x in bf16 as the stationary operand,
    W hi/lo bf16-split as a 2-column moving operand, accumulated over
    the two 128-row halves of node_dim.
  - Output is stored [128, 576] per core (main cols 0..431 by tile id,
    folded cols 432..575 by group id); host un-permutes.
"""

import sys

if "/opt/trn_rl_repo" not in sys.path:
    sys.path.insert(0, "/opt/trn_rl_repo")

import ml_dtypes
import numpy as np

import concourse.bass as bass  # noqa: F401
import concourse.mybir as mybir
from concourse import bacc, bass_utils
from concourse.tile import TileContext

F32 = mybir.dt.float32
BF16 = mybir.dt.bfloat16
FP8 = mybir.dt.float8e4
AF = mybir.ActivationFunctionType

B = 64
MAX_A = 96
D = 256
N_CORES = 8
G = B // N_CORES            # 8 graphs per core
COLS = MAX_A * MAX_A        # 9216
CPG = COLS // 128           # 72 chunks per graph
GPG = CPG // 4              # 18 fold-groups per graph
TPG = 3 * GPG               # 54 stationary tiles per graph
TT = G * TPG                # 432 tiles per core
TPB = 72                    # tiles per PSUM bank (72*6 = 432 f32 cols)
NB = TT // TPB              # 6 banks
QPB = TPB // 3              # 24 fold-groups per bank
EPS = 1e-8
LO_SCALE = 0.0625           # sp_lo carries 16x the residual

_NC_CACHE = {}


def _build():
    if "nc" in _NC_CACHE:
        return _NC_CACHE["nc"]

    nc = bacc.Bacc("TRN2", target_bir_lowering=False, debug=False,
                   num_devices=N_CORES)

    pathsT_d = nc.dram_tensor("pathsT", [G, 128, TPG * 128], FP8,
                              kind="ExternalInput")
    xt_d = nc.dram_tensor("xt", [128, 2 * G * MAX_A], BF16,
                          kind="ExternalInput")
    wmov_d = nc.dram_tensor("wmov", [128, 4], BF16, kind="ExternalInput")
    bmask_d = nc.dram_tensor("bmask", [MAX_A, G], F32, kind="ExternalInput")
    out_d = nc.dram_tensor("out", [128, TT + NB * QPB], F32,
                           kind="ExternalOutput")

    with TileContext(nc) as tc:
        with (
            tc.tile_pool(name="misc", bufs=1) as misc,
            tc.tile_pool(name="psum", bufs=1, space="PSUM") as pspool,
        ):
            # small score inputs ride the scalar (ACT) HWDGE ring so they
            # overlap the big paths DMAs on the sync ring
            xt = misc.tile([128, 2 * G * MAX_A], BF16)
            nc.scalar.dma_start(out=xt[:], in_=xt_d[:])
            wmov = misc.tile([128, 4], BF16)
            nc.scalar.dma_start(out=wmov[:], in_=wmov_d[:])
            bm = misc.tile([MAX_A, G], F32)
            nc.scalar.dma_start(out=bm[:], in_=bmask_d[:])

            paths_sb = []
            for g in range(G):
                st = misc.tile([128, TPG * 128], FP8, name=f"st{g}")
                nc.sync.dma_start(out=st[:], in_=pathsT_d[g])
                paths_sb.append(st)

            # ---- node scores on the PE ----
            ps_s = pspool.tile([MAX_A, 2 * G], F32, name="ps_scores")
            for g in range(G):
                for h in range(2):
                    nc.tensor.matmul(
                        ps_s[:, 2 * g:2 * g + 2],
                        lhsT=xt[:, G * MAX_A * h + MAX_A * g:
                                G * MAX_A * h + MAX_A * (g + 1)],
                        rhs=wmov[:, 2 * h:2 * h + 2],
                        start=(h == 0), stop=(h == 1))

            s_ev = misc.tile([MAX_A, G], F32)
            nc.scalar.activation(out=s_ev[:], in_=ps_s[:, 0:2 * G:2],
                                 func=AF.Copy)
            w_tmp = misc.tile([MAX_A, G], F32)
            nc.vector.tensor_tensor(out=w_tmp[:], in0=ps_s[:, 1:2 * G:2],
                                    in1=s_ev[:], op=mybir.AluOpType.add)
            w_sp = misc.tile([MAX_A, G], F32)
            nc.vector.tensor_tensor(out=w_sp[:], in0=w_tmp[:], in1=bm[:],
                                    op=mybir.AluOpType.add)

            # sp -> fp8 hi + fp8 lo (residual x16)
            hi8 = misc.tile([MAX_A, G], FP8)
            nc.vector.tensor_copy(hi8[:], w_sp[:])
            r1 = misc.tile([MAX_A, G], F32)
            nc.vector.tensor_tensor(out=r1[:], in0=w_sp[:], in1=hi8[:],
                                    op=mybir.AluOpType.subtract)
            lo8 = misc.tile([MAX_A, G], FP8)
            nc.vector.tensor_scalar_mul(out=lo8[:], in0=r1[:], scalar1=16.0)

            # ---- moving operand w_all [128, 18G]: per (g,t) 6 cols
            #      [mh, ml, md, ph, pl, pd] at 18g+6t ----
            WCOLS = 18 * G
            w_all = misc.tile([128, WCOLS], FP8)
            nc.vector.memset(w_all[:], 0.0)
            nc.vector.memset(w_all[0:MAX_A, 2:WCOLS:6], 1.0)
            nc.vector.memset(w_all[MAX_A:128, 5:WCOLS:6], 1.0)
            for t in range(3):
                nc.vector.tensor_copy(w_all[0:MAX_A, 6 * t:WCOLS:18],
                                      hi8[:])
                nc.vector.tensor_copy(w_all[0:MAX_A, 6 * t + 1:WCOLS:18],
                                      lo8[:])
                nc.vector.tensor_copy(w_all[MAX_A:128, 6 * t + 3:WCOLS:18],
                                      hi8[32 * t:32 * (t + 1), :])
                nc.vector.tensor_copy(w_all[MAX_A:128, 6 * t + 4:WCOLS:18],
                                      lo8[32 * t:32 * (t + 1), :])

            out_sb = misc.tile([128, TT + NB * QPB], F32)

            # ---- main loop: 432 matmuls, epilogue per PSUM bank ----
            pss = [pspool.tile([128, 6 * TPB], F32, name=f"ps{b}")
                   for b in range(NB)]
            for tile in range(TT):
                g, tig = divmod(tile, TPG)
                t = tig % 3
                b, j = divmod(tile, TPB)
                nc.tensor.matmul(
                    pss[b][:, 6 * j:6 * j + 6],
                    lhsT=paths_sb[g][:, 128 * tig:128 * (tig + 1)],
                    rhs=w_all[:, 18 * g + 6 * t:18 * g + 6 * t + 6],
                    start=True, stop=True)
                if j == TPB - 1:
                    _epilogue(nc, misc, pss[b], out_sb, b)

            nc.sync.dma_start(out=out_d[:], in_=out_sb[:])

    nc.compile()
    _NC_CACHE["nc"] = nc
    return nc


def _epilogue(nc, misc, ps, out_sb, b):
    """Reduce one PSUM bank (72 tiles = 24 fold groups) to outputs."""
    W = 6 * TPB
    # folded partials: sum t=0..2 for each (group, var) in one reduce
    rvar = misc.tile([128, 3 * QPB], F32, name=f"rvar{b}")
    nc.vector.tensor_reduce(
        out=rvar[:],
        in_=ps[:].rearrange("p (q t v) -> p q v t", t=3, v=6)[:, :, 3:6, :],
        axis=mybir.AxisListType.X, op=mybir.AluOpType.add)
    t1d = misc.tile([128, QPB], F32, name=f"t1d{b}")
    nc.vector.scalar_tensor_tensor(
        out=t1d[:], in0=rvar[:, 1:3 * QPB:3], scalar=LO_SCALE,
        in1=rvar[:, 0:3 * QPB:3],
        op0=mybir.AluOpType.mult, op1=mybir.AluOpType.add)
    dend = misc.tile([128, QPB], F32, name=f"dend{b}")
    nc.scalar.activation(out=dend[:], in_=rvar[:, 2:3 * QPB:3],
                         func=AF.Copy, bias=EPS)
    recd = misc.tile([128, QPB], F32, name=f"recd{b}")
    nc.vector.reciprocal(out=recd[:], in_=dend[:])
    nc.vector.tensor_tensor(
        out=out_sb[:, TT + QPB * b:TT + QPB * (b + 1)],
        in0=t1d[:], in1=recd[:], op=mybir.AluOpType.mult)

    # main chunks
    him = misc.tile([128, TPB], F32, name=f"him{b}")
    nc.scalar.activation(out=him[:], in_=ps[:, 0:W:6], func=AF.Copy)
    t1m = misc.tile([128, TPB], F32, name=f"t1m{b}")
    nc.vector.scalar_tensor_tensor(
        out=t1m[:], in0=ps[:, 1:W:6], scalar=LO_SCALE, in1=him[:],
        op0=mybir.AluOpType.mult, op1=mybir.AluOpType.add)
    denm = misc.tile([128, TPB], F32, name=f"denm{b}")
    nc.scalar.activation(out=denm[:], in_=ps[:, 2:W:6],
                         func=AF.Copy, bias=EPS)
    recm = misc.tile([128, TPB], F32, name=f"recm{b}")
    nc.vector.reciprocal(out=recm[:], in_=denm[:])
    nc.vector.tensor_tensor(
        out=out_sb[:, TPB * b:TPB * (b + 1)],
        in0=t1m[:], in1=recm[:], op=mybir.AluOpType.mult)


def _host_prep(x, W, b, paths, pad_idx):
    x = np.ascontiguousarray(np.asarray(x, dtype=np.float32))
    W = np.asarray(W, dtype=np.float32).reshape(D)
    b = np.asarray(b, dtype=np.float32)
    pad_idx = np.asarray(pad_idx)

    xsc = np.zeros((B * MAX_A, D), dtype=np.float32)
    xsc[pad_idx] = x
    valid = np.zeros((B * MAX_A,), dtype=np.float32)
    valid[pad_idx] = 1.0
    bmask_full = (b[0] * valid).reshape(B, MAX_A)

    whi = W.astype(ml_dtypes.bfloat16)
    wlo = (W - whi.astype(np.float32)).astype(ml_dtypes.bfloat16)
    wmov = np.zeros((128, 4), dtype=ml_dtypes.bfloat16)
    wmov[:, 0] = whi[0:128]
    wmov[:, 1] = wlo[0:128]
    wmov[:, 2] = whi[128:256]
    wmov[:, 3] = wlo[128:256]

    paths_f8 = np.asarray(paths).astype(ml_dtypes.float8_e4m3)

    in_maps = []
    for core in range(N_CORES):
        g0 = core * G
        # fold-32 pack: [g, k, q, s(chunk-in-group), c]
        PT = paths_f8[g0:g0 + G].transpose(0, 3, 1, 2).reshape(
            G, MAX_A, GPG, 4, 128)
        A = np.empty((G, 128, GPG, 3, 128), dtype=ml_dtypes.float8_e4m3)
        A[:, 0:MAX_A] = PT[:, :, :, 0:3, :]
        for t in range(3):
            A[:, MAX_A:128, :, t, :] = PT[:, 32 * t:32 * (t + 1), :, 3, :]
        pathsT = np.ascontiguousarray(A.reshape(G, 128, TPG * 128))

        xc = xsc[g0 * MAX_A:(g0 + G) * MAX_A].reshape(G, MAX_A, D)
        # xt[p, G*MAX_A*h + MAX_A*g + k] = x[g, k, 128h + p]
        xt = np.ascontiguousarray(
            xc.transpose(2, 0, 1).reshape(2, 128, G * MAX_A)
            .transpose(1, 0, 2).reshape(128, 2 * G * MAX_A)
        ).astype(ml_dtypes.bfloat16)
        bmask = np.ascontiguousarray(bmask_full[g0:g0 + G].T)
        in_maps.append({
            "pathsT": pathsT,
            "xt": xt,
            "wmov": wmov,
            "bmask": bmask,
        })
    return in_maps


def _out_perm():
    """col -> (graph, chunk) for the [128, 576] per-core output."""
    g_arr = np.empty(TT + NB * QPB, dtype=np.int64)
    c_arr = np.empty(TT + NB * QPB, dtype=np.int64)
    for col in range(TT):
        g, tig = divmod(col, TPG)
        q, t = divmod(tig, 3)
        g_arr[col], c_arr[col] = g, 4 * q + t
    for dcol in range(NB * QPB):
        grp = dcol  # global fold-group id = 24*b + q_in_bank = sequential
        g, q = divmod(grp, GPG)
        g_arr[TT + dcol], c_arr[TT + dcol] = g, 4 * q + 3
    return g_arr, c_arr


_G_ARR, _C_ARR = _out_perm()

LAST_RESULTS = None


def kernel(x, W, b, paths, pad_idx, _trace=False):
    global LAST_RESULTS
    nc = _build()
    in_maps = _host_prep(x, W, b, paths, pad_idx)
    res = bass_utils.run_bass_kernel_spmd(
        nc, in_maps, core_ids=list(range(N_CORES)), trace=_trace)
    LAST_RESULTS = res

    out = np.empty((B, MAX_A, MAX_A), dtype=np.float32)
    for core in range(N_CORES):
        oc = res.results[core]["out"]  # [128, 576]
        o3 = np.empty((G, CPG, 128), dtype=np.float32)
        o3[_G_ARR, _C_ARR] = oc.T
        out[core * G:(core + 1) * G] = o3.reshape(G, MAX_A, MAX_A)
    return out


# revision 7
# speedup vs baseline: 1.5370x; 1.1536x over previous
"""Trainium2 Bass kernel for nn_NodeEncoding_72816875537095.

Reference computation:
    scores = x @ W[0] + b[0]                          # [total]
    sp     = scatter(scores, pad_idx) -> [B, 96]      # padded per-graph scores
    num    = einsum('bijk,bk->bij', paths, sp)
    den    = paths.sum(-1) + 1e-8
    out    = num / den                                # [64, 96, 96]

Strategy (data-parallel over B across 8 NeuronCores, 8 graphs/core):
  - paths (0/1 valued) are exact in fp8e4m3.  Per graph, the k-major
    [96, 9216] matrix is cut into 72 column-chunks of 128.  "Fold-32"
    packing: chunks are grouped 4-per-3-tiles; stationary tile t of a
    group holds chunk (4q+t) k-rows on partitions 0..95 AND a 32-row
    k-slice [32t, 32t+32) of chunk (4q+3) on partitions 96..127.  Every
    PE cell carries real data: 432 ldweights/core (the minimum) and
    7.08 MB of HBM traffic/core (no zero padding).
  - Moving operand per tile: 6 fp8 columns
        [sp_hi, sp_lo, ones (rows 0..95) | fold sp_hi, sp_lo, ones
         (rows 96..127, k-slice t of the folded chunk)]
    -> PSUM [128, 6] = main (num_hi, num_lo, den) + folded partials.
    sp is hi/lo fp8-split (8+ mantissa bits); folded chunks sum their 3
    partials with one DVE tensor_reduce per psum bank.
  - node scores run on the PE too: x in bf16 as the stationary operand,
    W hi/lo bf16-split as a 2-column moving operand, accumulated over
    the two 128-row halves of node_dim.  b is added to every padded
    slot (valid or not): paths' k-mask zeroes invalid contributions.
  - DMA order matters: xt leads the sync queue (so it is not starved by
    the paths stream), paths follow in half-graph slices, outputs leave
    in per-2-bank slices as epilogues complete.  wmov/bmask ride the
    scalar ring.
  - Output is stored [128, 576] per core as 6 bank-blocks of
    [72 main | 24 folded] columns; host un-permutes.
"""

import sys

if "/opt/trn_rl_repo" not in sys.path:
    sys.path.insert(0, "/opt/trn_rl_repo")

import ml_dtypes
import numpy as np

import concourse.bass as bass  # noqa: F401
import concourse.mybir as mybir
from concourse import bacc, bass_utils
from concourse.tile import TileContext

F32 = mybir.dt.float32
BF16 = mybir.dt.bfloat16
FP8 = mybir.dt.float8e4
AF = mybir.ActivationFunctionType

B = 64
MAX_A = 96
D = 256
N_CORES = 8
G = B // N_CORES            # 8 graphs per core
COLS = MAX_A * MAX_A        # 9216
CPG = COLS // 128           # 72 chunks per graph
GPG = CPG // 4              # 18 fold-groups per graph
TPG = 3 * GPG               # 54 stationary tiles per graph
TT = G * TPG                # 432 tiles per core
TPB = 72                    # tiles per PSUM bank (72*6 = 432 f32 cols)
NB = TT // TPB              # 6 banks
QPB = TPB // 3              # 24 fold-groups per bank
OPB = TPB + QPB             # 96 output cols per bank
EPS = 1e-8
LO_SCALE = 0.0625           # sp_lo carries 16x the residual

_NC_CACHE = {}


def _build():
    if "nc" in _NC_CACHE:
        return _NC_CACHE["nc"]

    nc = bacc.Bacc("TRN2", target_bir_lowering=False, debug=False,
                   num_devices=N_CORES)

    pathsT_d = nc.dram_tensor("pathsT", [G, 128, TPG * 128], FP8,
                              kind="ExternalInput")
    xt_d = nc.dram_tensor("xt", [128, 2 * G * MAX_A], BF16,
                          kind="ExternalInput")
    wmov_d = nc.dram_tensor("wmov", [128, 4], BF16, kind="ExternalInput")
    bmask_d = nc.dram_tensor("bmask", [MAX_A, G], F32, kind="ExternalInput")
    out_d = nc.dram_tensor("out", [128, NB * OPB], F32,
                           kind="ExternalOutput")

    HALF = TPG * 128 // 2

    with TileContext(nc) as tc:
        with (
            tc.tile_pool(name="misc", bufs=1) as misc,
            tc.tile_pool(name="psum", bufs=1, space="PSUM") as pspool,
        ):
            # xt leads the sync ring; paths follow in half-graph slices.
            xt = misc.tile([128, 2 * G * MAX_A], BF16)
            nc.sync.dma_start(out=xt[:], in_=xt_d[:])
            paths_sb = []
            for g in range(G):
                st = misc.tile([128, TPG * 128], FP8, name=f"st{g}")
                nc.sync.dma_start(out=st[:, 0:HALF],
                                  in_=pathsT_d[g, :, 0:HALF])
                nc.sync.dma_start(out=st[:, HALF:2 * HALF],
                                  in_=pathsT_d[g, :, HALF:2 * HALF])
                paths_sb.append(st)

            wmov = misc.tile([128, 4], BF16)
            nc.scalar.dma_start(out=wmov[:], in_=wmov_d[:])
            bm = misc.tile([MAX_A, G], F32)
            nc.scalar.dma_start(out=bm[:], in_=bmask_d[:])

            # ---- node scores on the PE ----
            ps_s = pspool.tile([MAX_A, 2 * G], F32, name="ps_scores")
            for g in range(G):
                for h in range(2):
                    nc.tensor.matmul(
                        ps_s[:, 2 * g:2 * g + 2],
                        lhsT=xt[:, G * MAX_A * h + MAX_A * g:
                                G * MAX_A * h + MAX_A * (g + 1)],
                        rhs=wmov[:, 2 * h:2 * h + 2],
                        start=(h == 0), stop=(h == 1))

            s_ev = misc.tile([MAX_A, G], F32)
            nc.scalar.activation(out=s_ev[:], in_=ps_s[:, 0:2 * G:2],
                                 func=AF.Copy)
            w_tmp = misc.tile([MAX_A, G], F32)
            nc.vector.tensor_tensor(out=w_tmp[:], in0=ps_s[:, 1:2 * G:2],
                                    in1=s_ev[:], op=mybir.AluOpType.add)
            w_sp = misc.tile([MAX_A, G], F32)
            nc.vector.tensor_tensor(out=w_sp[:], in0=w_tmp[:], in1=bm[:],
                                    op=mybir.AluOpType.add)

            # sp -> fp8 hi + fp8 lo (residual x16), interleaved [hi lo]
            hl = misc.tile([MAX_A, 2 * G], FP8)
            nc.vector.tensor_copy(hl[:, 0:2 * G:2], w_sp[:])
            r1 = misc.tile([MAX_A, G], F32)
            nc.vector.tensor_tensor(out=r1[:], in0=w_sp[:],
                                    in1=hl[:, 0:2 * G:2],
                                    op=mybir.AluOpType.subtract)
            nc.vector.tensor_scalar_mul(out=hl[:, 1:2 * G:2], in0=r1[:],
                                        scalar1=16.0)

            # ---- moving operand w_all [128, 18G]: per (g,t) 6 cols
            #      [mh, ml, md, ph, pl, pd] at 18g+6t ----
            WCOLS = 18 * G
            w_all = misc.tile([128, WCOLS], FP8)
            nc.vector.memset(w_all[:], 0.0)
            nc.vector.memset(w_all[0:MAX_A, 2:WCOLS:6], 1.0)
            nc.vector.memset(w_all[MAX_A:128, 5:WCOLS:6], 1.0)
            wa_top = w_all[0:MAX_A, :].rearrange("p (g v) -> p g v", v=18)
            wa_bot = w_all[MAX_A:128, :].rearrange("p (g v) -> p g v", v=18)
            for t in range(3):
                nc.vector.tensor_copy(
                    wa_top[:, :, 6 * t:6 * t + 2],
                    hl[:].rearrange("p (g c) -> p g c", c=2))
                nc.vector.tensor_copy(
                    wa_bot[:, :, 6 * t + 3:6 * t + 5],
                    hl[32 * t:32 * (t + 1), :]
                    .rearrange("p (g c) -> p g c", c=2))

            out_sb = misc.tile([128, NB * OPB], F32)

            # ---- main loop: 432 matmuls, epilogue per PSUM bank ----
            pss = [pspool.tile([128, 6 * TPB], F32, name=f"ps{b}")
                   for b in range(NB)]
            for tile in range(TT):
                g, tig = divmod(tile, TPG)
                t = tig % 3
                b, j = divmod(tile, TPB)
                nc.tensor.matmul(
                    pss[b][:, 6 * j:6 * j + 6],
                    lhsT=paths_sb[g][:, 128 * tig:128 * (tig + 1)],
                    rhs=w_all[:, 18 * g + 6 * t:18 * g + 6 * t + 6],
                    start=True, stop=True)
                if j == TPB - 1:
                    _epilogue(nc, misc, pss[b], out_sb, b)
                    if b % 2 == 1:
                        c0 = OPB * (b - 1)
                        c1 = OPB * (b + 1)
                        nc.sync.dma_start(out=out_d[:, c0:c1],
                                          in_=out_sb[:, c0:c1])

    nc.compile()
    _NC_CACHE["nc"] = nc
    return nc


def _epilogue(nc, misc, ps, out_sb, b):
    """Reduce one PSUM bank (72 tiles = 24 fold groups) to outputs."""
    W = 6 * TPB
    o0 = OPB * b
    den_all = misc.tile([128, OPB], F32, name=f"den{b}")
    # main den only needs psum: overlaps the fold reduce on DVE
    nc.scalar.activation(out=den_all[:, 0:TPB], in_=ps[:, 2:W:6],
                         func=AF.Copy, bias=EPS)
    him = misc.tile([128, TPB], F32, name=f"him{b}")
    nc.scalar.activation(out=him[:], in_=ps[:, 0:W:6], func=AF.Copy)

    # folded partials: sum t=0..2 for each (group, var) in one reduce
    rvar = misc.tile([128, 3 * QPB], F32, name=f"rvar{b}")
    nc.vector.tensor_reduce(
        out=rvar[:],
        in_=ps[:].rearrange("p (q t v) -> p q v t", t=3, v=6)[:, :, 3:6, :],
        axis=mybir.AxisListType.X, op=mybir.AluOpType.add)
    nc.scalar.activation(out=den_all[:, TPB:OPB], in_=rvar[:, 2:3 * QPB:3],
                         func=AF.Copy, bias=EPS)
    rec = misc.tile([128, OPB], F32, name=f"rec{b}")
    nc.vector.reciprocal(out=rec[:], in_=den_all[:])

    t1d = misc.tile([128, QPB], F32, name=f"t1d{b}")
    nc.vector.scalar_tensor_tensor(
        out=t1d[:], in0=rvar[:, 1:3 * QPB:3], scalar=LO_SCALE,
        in1=rvar[:, 0:3 * QPB:3],
        op0=mybir.AluOpType.mult, op1=mybir.AluOpType.add)
    t1m = misc.tile([128, TPB], F32, name=f"t1m{b}")
    nc.vector.scalar_tensor_tensor(
        out=t1m[:], in0=ps[:, 1:W:6], scalar=LO_SCALE, in1=him[:],
        op0=mybir.AluOpType.mult, op1=mybir.AluOpType.add)

    nc.vector.tensor_tensor(
        out=out_sb[:, o0:o0 + TPB],
        in0=t1m[:], in1=rec[:, 0:TPB], op=mybir.AluOpType.mult)
    nc.vector.tensor_tensor(
        out=out_sb[:, o0 + TPB:o0 + OPB],
        in0=t1d[:], in1=rec[:, TPB:OPB], op=mybir.AluOpType.mult)


def _host_prep(x, W, b, paths, pad_idx):
    x = np.ascontiguousarray(np.asarray(x, dtype=np.float32))
    W = np.asarray(W, dtype=np.float32).reshape(D)
    b = np.asarray(b, dtype=np.float32)
    pad_idx = np.asarray(pad_idx)

    xsc = np.zeros((B * MAX_A, D), dtype=np.float32)
    xsc[pad_idx] = x
    valid = np.zeros((B * MAX_A,), dtype=np.float32)
    valid[pad_idx] = 1.0
    bmask_full = (b[0] * valid).reshape(B, MAX_A)

    whi = W.astype(ml_dtypes.bfloat16)
    wlo = (W - whi.astype(np.float32)).astype(ml_dtypes.bfloat16)
    wmov = np.zeros((128, 4), dtype=ml_dtypes.bfloat16)
    wmov[:, 0] = whi[0:128]
    wmov[:, 1] = wlo[0:128]
    wmov[:, 2] = whi[128:256]
    wmov[:, 3] = wlo[128:256]

    paths_f8 = np.asarray(paths).astype(ml_dtypes.float8_e4m3)

    in_maps = []
    for core in range(N_CORES):
        g0 = core * G
        # fold-32 pack: [g, k, q, s(chunk-in-group), c]
        PT = paths_f8[g0:g0 + G].transpose(0, 3, 1, 2).reshape(
            G, MAX_A, GPG, 4, 128)
        A = np.empty((G, 128, GPG, 3, 128), dtype=ml_dtypes.float8_e4m3)
        A[:, 0:MAX_A] = PT[:, :, :, 0:3, :]
        for t in range(3):
            A[:, MAX_A:128, :, t, :] = PT[:, 32 * t:32 * (t + 1), :, 3, :]
        pathsT = np.ascontiguousarray(A.reshape(G, 128, TPG * 128))

        xc = xsc[g0 * MAX_A:(g0 + G) * MAX_A].reshape(G, MAX_A, D)
        # xt[p, G*MAX_A*h + MAX_A*g + k] = x[g, k, 128h + p]
        xt = np.ascontiguousarray(
            xc.transpose(2, 0, 1).reshape(2, 128, G * MAX_A)
            .transpose(1, 0, 2).reshape(128, 2 * G * MAX_A)
        ).astype(ml_dtypes.bfloat16)
        bmask = np.ascontiguousarray(bmask_full[g0:g0 + G].T)
        in_maps.append({
            "pathsT": pathsT,
            "xt": xt,
            "wmov": wmov,
            "bmask": bmask,
        })
    return in_maps


def _out_perm():
    """col -> (graph, chunk) for the [128, 576] per-core output."""
    n = NB * OPB
    g_arr = np.empty(n, dtype=np.int64)
    c_arr = np.empty(n, dtype=np.int64)
    for col in range(n):
        bank, j = divmod(col, OPB)
        if j < TPB:
            tile = TPB * bank + j
            g, tig = divmod(tile, TPG)
            q, t = divmod(tig, 3)
            g_arr[col], c_arr[col] = g, 4 * q + t
        else:
            grp = QPB * bank + (j - TPB)
            g, q = divmod(grp, GPG)
            g_arr[col], c_arr[col] = g, 4 * q + 3
    return g_arr, c_arr


_G_ARR, _C_ARR = _out_perm()

LAST_RESULTS = None


def kernel(x, W, b, paths, pad_idx, _trace=False):
    global LAST_RESULTS
    nc = _build()
    in_maps = _host_prep(x, W, b, paths, pad_idx)
    res = bass_utils.run_bass_kernel_spmd(
        nc, in_maps, core_ids=list(range(N_CORES)), trace=_trace)
    LAST_RESULTS = res

    out = np.empty((B, MAX_A, MAX_A), dtype=np.float32)
    for core in range(N_CORES):
        oc = res.results[core]["out"]  # [128, 576]
        o3 = np.empty((G, CPG, 128), dtype=np.float32)
        o3[_G_ARR, _C_ARR] = oc.T
        out[core * G:(core + 1) * G] = o3.reshape(G, MAX_A, MAX_A)
    return out
